# revision 54
# baseline (speedup 1.0000x reference)
"""GQA attention (B=2,S=2048,E=2048,H=32,KVH=8,D=64, RoPE, non-causal) on 8 TRN2 cores.

Sharding: core = 4*b + g  (b = batch, g = head-group).  Each core owns one batch
and 8 q-heads / 2 kv-heads, computes a partial output projection; host sums the
4 group partials per batch.

v2 pipeline (vs the v1 baseline):
  - scores matmuls are row-tiled: the two 64-row halves of the PE array run two
    heads' K=64 contractions concurrently (2x effective scores throughput).
  - V is projected feature-major (16 LDW instead of 256) and transposed to
    seq-major on the PE.
  - exp() runs on [128,1024] PSUM tiles (half the instruction count), and the
    softmax denominator is handled per 2-head round with reciprocal_approx_fast
    at partition 64 plus a K=1 broadcast matmul (replaces 32 x 3.4us DVE
    reciprocals).
  - loop nests keep the PE dense (no >3.4us idle windows) so the HAM clock
    gate stays at 2.4 GHz.
"""

import numpy as np
import ml_dtypes

import concourse.bass as bass
import concourse.tile as tile
from concourse import bacc, mybir
from concourse.bass_utils import run_bass_kernel_spmd

BF16 = ml_dtypes.bfloat16
F32 = mybir.dt.float32
BF = mybir.dt.bfloat16

B, S, E = 2, 2048, 2048
H, KVH, D = 32, 8, 64
N_CORES = 8
FH = 512          # q features per core (8 heads * 64)
EC = 16           # 128-row chunks of E

_CACHE = {}


def _build():
    nc = bacc.Bacc("TRN2", target_bir_lowering=False, debug=False,
                   num_devices=N_CORES)
    xt_d = nc.dram_tensor("xt", [E, S], BF, kind="ExternalInput")
    wqt_d = nc.dram_tensor("wqt", [E, FH], BF, kind="ExternalInput")
    wkt_d = nc.dram_tensor("wkt", [E, 128], BF, kind="ExternalInput")
    wvt_d = nc.dram_tensor("wvt", [E, 128], BF, kind="ExternalInput")
    wot_d = nc.dram_tensor("wot", [FH, E], BF, kind="ExternalInput")
    cos_d = nc.dram_tensor("cost", [128, S], BF, kind="ExternalInput")
    nsin_d = nc.dram_tensor("nsint", [128, S], BF, kind="ExternalInput")
    id_d = nc.dram_tensor("ident", [128, 128], BF, kind="ExternalInput")
    out_d = nc.dram_tensor("out", [S, E], F32, kind="ExternalOutput")

    EXP = mybir.ActivationFunctionType.Exp

    from contextlib import ExitStack
    with ExitStack() as ctx:
        tc = ctx.enter_context(tile.TileContext(nc))
        pool = lambda *a, **k: ctx.enter_context(tc.tile_pool(*a, **k))
        xt_p = pool(name="xt", bufs=16)
        wqt_p = pool(name="wqt", bufs=16)
        wkt_p = pool(name="wkt", bufs=16)
        wvt_p = pool(name="wvt", bufs=16)
        wot_p = pool(name="wot", bufs=4)
        cs_p = pool(name="cs", bufs=2)
        id_p = pool(name="idp", bufs=1)
        kraw_p = pool(name="kraw", bufs=1)
        t2_p = pool(name="t2", bufs=2)
        ktd_p = pool(name="ktd", bufs=2)
        qraw_p = pool(name="qraw", bufs=1)
        qt_p = pool(name="qt", bufs=4)
        vtT_p = pool(name="vtT", bufs=1)
        vt_p = pool(name="vt", bufs=32)
        pt_p = pool(name="pt", bufs=4)
        attnt_p = pool(name="attnt", bufs=4)
        praw_p = pool(name="praw", bufs=4)
        rcp_p = pool(name="rcp", bufs=2)
        rcb_p = pool(name="rcb", bufs=2)
        nt_p = pool(name="nt", bufs=2)
        ones_p = pool(name="ones", bufs=1)
        ostage_p = pool(name="ostage", bufs=2)
        ps = pool(name="ps", bufs=2, space="PSUM")

        # ---- input DMA (order = arrival priority) ----
        xt, wkt, wqt = [], [], []
        for i in range(EC):
            t = wkt_p.tile([128, 128], BF, tag="wkt", name=f"wkt{i}")
            nc.sync.dma_start(t[:], wkt_d[128 * i:128 * (i + 1), :])
            wkt.append(t)
            t = xt_p.tile([128, S], BF, tag="xt", name=f"xt{i}")
            nc.sync.dma_start(t[:], xt_d[128 * i:128 * (i + 1), :])
            xt.append(t)
            t = wqt_p.tile([128, FH], BF, tag="wqt", name=f"wqt{i}")
            nc.sync.dma_start(t[:], wqt_d[128 * i:128 * (i + 1), :])
            wqt.append(t)
        wvt = []
        for i in range(EC):
            t = wvt_p.tile([128, 128], BF, tag="wvt", name=f"wvt{i}")
            nc.sync.dma_start(t[:], wvt_d[128 * i:128 * (i + 1), :])
            wvt.append(t)
        cos_t = cs_p.tile([128, S], BF, tag="cs")
        nc.sync.dma_start(cos_t[:], cos_d[:, :])
        nsin_t = cs_p.tile([128, S], BF, tag="cs")
        nc.sync.dma_start(nsin_t[:], nsin_d[:, :])
        ident_t = id_p.tile([128, 128], BF, tag="id")
        nc.sync.dma_start(ident_t[:], id_d[:, :])
        wot = []
        for i in range(4):
            t = wot_p.tile([128, E], BF, tag="wot", name=f"wot{i}")
            nc.sync.dma_start(t[:], wot_d[128 * i:128 * (i + 1), :])
            wot.append(t)

        ones_t = ones_p.tile([65, 64], BF, tag="ones")
        nc.vector.memset(ones_t[64:65, :], 1.0)

        def proj4(w_tiles, wsl, dst, dst_sl=None):
            """dst[:, :] = (sum_ec w[ec][:, wsl].T @ xt[ec]) as bf16 via 4
            psum tiles (2 from tag mm, 2 from tag sc)."""
            t4 = [ps.tile([128, 512], F32, tag="mm", name="pj0"),
                  ps.tile([128, 512], F32, tag="mm", name="pj1"),
                  ps.tile([128, 512], F32, tag="sc", name="pj2"),
                  ps.tile([128, 512], F32, tag="sc", name="pj3")]
            for ec in range(EC):
                for j in range(4):
                    nc.tensor.matmul(t4[j][:], w_tiles[ec][:, wsl],
                                     xt[ec][:, 512 * j:512 * (j + 1)],
                                     start=(ec == 0), stop=(ec == EC - 1))
            for j in range(4):
                nc.vector.tensor_copy(dst[:, 512 * j:512 * (j + 1)], t4[j][:])

        def rope(src, dst):
            # dst = src*cos + shift32(src)*nsin, per 64-row head block.
            # partition shift must go through DMA (engines are lane-locked)
            qs = t2_p.tile([128, S], BF, tag="t2", name="qs")
            for blk in (0, 64):
                nc.sync.dma_start(qs[blk:blk + 32, :],
                                  src[blk + 32:blk + 64, :])
                nc.sync.dma_start(qs[blk + 32:blk + 64, :],
                                  src[blk:blk + 32, :])
            t2 = t2_p.tile([128, S], BF, tag="t2", name="t2")
            nc.vector.tensor_mul(t2[:], qs[:], nsin_t[:])
            nc.vector.tensor_mul(dst[:], src[:], cos_t[:])
            nc.vector.tensor_add(dst[:], dst[:], t2[:])

        # ---- K (sc psum ring) + Q0 (mm+pv rings) interleaved per-ec so
        # both projections ride the DMA-paced window instead of Q0 running
        # 20us serially after K ----
        ktps = [ps.tile([128, 1024], F32, tag="sc", name=f"ktps{j}")
                for j in range(2)]
        q0ps = [ps.tile([128, 512], F32, tag=t, name=f"q0ps{j}")
                for j, t in enumerate(("mm", "mm", "pv", "pv"))]
        for ec in range(EC):
            st_, sp_ = (ec == 0), (ec == EC - 1)
            for j in range(4):
                nc.tensor.matmul(
                    ktps[j // 2][:, 512 * (j % 2):512 * (j % 2 + 1)],
                    wkt[ec][:, 0:128],
                    xt[ec][:, 512 * j:512 * (j + 1)], start=st_, stop=sp_)
            for j in range(4):
                nc.tensor.matmul(q0ps[j][:], wqt[ec][:, 0:128],
                                 xt[ec][:, 512 * j:512 * (j + 1)],
                                 start=st_, stop=sp_)
        kraw = kraw_p.tile([128, S], BF, tag="kraw")
        for j in range(2):
            nc.vector.tensor_copy(kraw[:, 1024 * j:1024 * (j + 1)],
                                  ktps[j][:])
        rope(kraw, kraw)
        ktd = [ktd_p.tile([128, S], BF, tag="ktd", name=f"ktd{i}")
               for i in range(2)]
        for kv in range(2):
            src = kraw[64 * kv:64 * (kv + 1), :]
            nc.sync.dma_start(ktd[kv][0:64, :], src)
            nc.sync.dma_start(ktd[kv][64:128, :], src)

        # ---- Q projections; head-pair f holds heads (2f, 2f+1) ----
        qt = [None] * 4

        def qproj(fc):
            qraw = qraw_p.tile([128, S], BF, tag="qraw")
            proj4(wqt, slice(128 * fc, 128 * (fc + 1)), qraw)
            qf = qt_p.tile([128, S], BF, tag="qt", name=f"qt{fc}")
            rope(qraw, qf)
            qt[fc] = qf

        qraw0 = qraw_p.tile([128, S], BF, tag="qraw", name="qraw0")
        for j in range(4):
            nc.vector.tensor_copy(qraw0[:, 512 * j:512 * (j + 1)], q0ps[j][:])
        qt0 = qt_p.tile([128, S], BF, tag="qt", name="qt0")
        rope(qraw0, qt0)
        qt[0] = qt0

        # ---- V^T projection (feature-major), then PE-transpose to seq-major
        vtT = vtT_p.tile([128, S], BF, tag="vtT")
        proj4(wvt, slice(0, 128), vtT)
        vt = {}  # (kc, kv) -> [128, 65] seq-major V with ones col
        for g in range(4):
            tr = ps.tile([128, 512], BF, tag="sc", name="tr")
            for j in range(4):
                kc = 4 * g + j
                nc.tensor.transpose(tr[:, 128 * j:128 * (j + 1)],
                                    vtT[:, 128 * kc:128 * (kc + 1)],
                                    ident_t[:])
            for j in range(4):
                kc = 4 * g + j
                for kv in range(2):
                    v = vt_p.tile([128, 65], BF, tag="vt", name=f"v{kc}_{kv}")
                    nc.vector.tensor_copy(
                        v[:, 0:64], tr[:, 128 * j + 64 * kv:128 * j + 64 * (kv + 1)])
                    nc.gpsimd.memset(v[:, 64:65], 1.0)
                    vt[(kc, kv)] = v

        # ---- attention ----
        attnt = [attnt_p.tile([128, S], BF, tag="attnt", name=f"attnt{i}")
                 for i in range(4)]

        def finish_norm(pend):
            # broadcast 1/den across partitions (K=1 matmul) and scale.
            # Deferred into the NEXT round's kc loop so the bc matmul never
            # head-of-line-blocks the PE queue while the reciprocal runs.
            f, qsl, info = pend
            for hl, praw, rcb in info:
                bc = ps.tile([64, 512], F32, tag="mm", name="bc")
                nc.tensor.matmul(bc[:], ones_t[64:65, 0:64],
                                 rcb[64:65, :], start=True, stop=True)
                if hl == 0:
                    nc.vector.tensor_mul(attnt[f][0:64, qsl], praw[:], bc[:])
                else:
                    ntt = nt_p.tile([64, 512], BF, tag="nt")
                    nc.vector.tensor_mul(ntt[:], praw[:], bc[:])
                    nc.sync.dma_start(attnt[f][64:128, qsl], ntt[:])

        def oproj_item(st, ep):
            # one (s-tile, e-half) chunk of the output projection
            ssl = slice(128 * st, 128 * (st + 1))
            ops = [ps.tile([128, 512], F32, tag="mm", name="op0"),
                   ps.tile([128, 512], F32, tag="mm", name="op1")]
            for fc2 in range(4):
                for j in range(2):
                    ecb = 2 * ep + j
                    nc.tensor.matmul(
                        ops[j][:], attnt[fc2][:, ssl],
                        wot[fc2][:, 512 * ecb:512 * (ecb + 1)],
                        start=(fc2 == 0), stop=(fc2 == 3))
            for j in range(2):
                ecb = 2 * ep + j
                so = ostage_p.tile([128, 512], F32, tag="ostage")
                nc.vector.tensor_copy(so[:], ops[j][:])
                nc.sync.dma_start(
                    out_d[ssl, 512 * ecb:512 * (ecb + 1)], so[:])

        def emit_sc(f, kv, qsl, kc):
            ksl = slice(128 * kc, 128 * (kc + 1))
            sp = ps.tile([128, 1024], F32, tag="sc", name="sp")
            nc.tensor.matmul(sp[:, 0:512], ktd[kv][0:64, ksl],
                             qt[f][0:64, qsl], start=True, stop=True)
            nc.tensor.matmul(sp[:, 512:1024], ktd[kv][64:128, ksl],
                             qt[f][64:128, qsl], start=True, stop=True)
            pt = pt_p.tile([128, 1024], BF, tag="pt")
            nc.scalar.activation(pt[:], sp[:], EXP, 0.0, 0.125)
            return pt

        rounds = [(qc, kv, r) for qc in range(4)
                  for kv in range(2) for r in range(2)]
        pending = None
        odue = []  # deferred output-projection items from the previous qc
        # bootstrap the prelude for round 0 so the ACT engine starts on the
        # first exps while round 0's qproj(1) monolith runs on the PE
        prelude = [emit_sc(0, 0, slice(0, 512), 0),
                   emit_sc(0, 0, slice(0, 512), 1)]
        for ridx, (qc, kv, r) in enumerate(rounds):
            qsl = slice(512 * qc, 512 * (qc + 1))
            f = 2 * kv + r
            if qc == 0 and f < 3 and qt[f + 1] is None:
                qproj(f + 1)  # keep PE busy hiding later q projections
            pva = ps.tile([128, 512], F32, tag="pv", name="pva")
            pvb = ps.tile([128, 512], F32, tag="pv", name="pvb")

            def pv_step(pkc, pt):
                nc.tensor.matmul(pva[0:65, :], vt[(pkc, kv)][:, 0:65],
                                 pt[:, 0:512],
                                 start=(pkc == 0), stop=(pkc == 15))
                nc.tensor.matmul(pvb[0:65, :], vt[(pkc, kv)][:, 0:65],
                                 pt[:, 512:1024],
                                 start=(pkc == 0), stop=(pkc == 15))

            pts = prelude if prelude is not None else []
            prelude = None
            for kc in range(len(pts), 16):
                pts.append(emit_sc(f, kv, qsl, kc))
                if kc == 2 and pending is not None:
                    finish_norm(pending)
                    pending = None
                if kc in (6, 12) and odue:
                    oproj_item(*odue.pop(0))
                if kc >= 2:
                    pv_step(kc - 2, pts[kc - 2])

            # pre-emit the next round's first two score/exp pairs so the ACT
            # engine isn't gapped while this round's last PV steps drain
            # (qt for the next round is always built by this point).
            if ridx + 1 < len(rounds):
                nqc, nkv, nr = rounds[ridx + 1]
                nf = 2 * nkv + nr
                nqsl = slice(512 * nqc, 512 * (nqc + 1))
                prelude = [emit_sc(nf, nkv, nqsl, 0),
                           emit_sc(nf, nkv, nqsl, 1)]

            pv_step(14, pts[14])
            pv_step(15, pts[15])

            # cheap per-round tail: reciprocal (full-tile custom DVE
            # op; partial-partition APs silently fail on hw) + praw
            # evacuation frees the pv psum ring quickly.
            info = []
            for hl, pv in ((0, pva), (1, pvb)):
                rcp = rcp_p.tile([128, 512], F32, tag="rcp")
                nc.vector.reciprocal_approx_fast(rcp[0:65, :], pv[0:65, :])
                praw = praw_p.tile([64, 512], BF, tag="praw")
                nc.vector.tensor_copy(praw[:], pv[0:64, :])
                rcb = rcb_p.tile([128, 512], BF, tag="rcb")
                nc.vector.tensor_copy(rcb[64:65, :], rcp[64:65, :])
                info.append((hl, praw, rcb))
            pending = (f, qsl, info)

            if (kv, r) == (1, 1):
                odue.extend((st, ep) for st in range(4 * qc, 4 * (qc + 1))
                            for ep in range(2))

        # ---- tail: last q-chunk's output projection. Keep the PE warm
        # through the final normalize chain (else HAM re-throttles and the
        # tail runs at half clock): fc2 0-2 don't depend on the final heads,
        # so issue them first, then the normalize, then fc2=3 + stores.
        def oproj_mm(ops, st, ep, fc2s):
            ssl = slice(128 * st, 128 * (st + 1))
            for fc2 in fc2s:
                for j in range(2):
                    ecb = 2 * ep + j
                    nc.tensor.matmul(ops[j][:], attnt[fc2][:, ssl],
                                     wot[fc2][:, 512 * ecb:512 * (ecb + 1)],
                                     start=(fc2 == 0), stop=(fc2 == 3))

        def oproj_out(ops, st, ep):
            ssl = slice(128 * st, 128 * (st + 1))
            for j in range(2):
                ecb = 2 * ep + j
                so = ostage_p.tile([128, 512], F32, tag="ostage")
                nc.vector.tensor_copy(so[:], ops[j][:])
                nc.sync.dma_start(
                    out_d[ssl, 512 * ecb:512 * (ecb + 1)], so[:])

        held_tags = ("pv", "sc")  # mm ring stays free for finish_norm's bc
        held = {}
        for idx in range(2):
            st, ep = odue[idx]
            ops = [ps.tile([128, 512], F32, tag=held_tags[idx],
                           name=f"to{idx}a"),
                   ps.tile([128, 512], F32, tag=held_tags[idx],
                           name=f"to{idx}b")]
            oproj_mm(ops, st, ep, (0, 1, 2))
            held[idx] = ops
        finish_norm(pending)
        pending = None
        tail_tags = ("mm", "sc")
        for idx, (st, ep) in enumerate(odue):
            if idx < 2:
                ops = held[idx]
                oproj_mm(ops, st, ep, (3,))
            else:
                ops = [ps.tile([128, 512], F32, tag=tail_tags[idx % 2],
                               name=f"to{idx}a"),
                       ps.tile([128, 512], F32, tag=tail_tags[idx % 2],
                               name=f"to{idx}b")]
                oproj_mm(ops, st, ep, (0, 1, 2, 3))
            oproj_out(ops, st, ep)

    nc.compile()
    return nc


def _tables():
    inv = 1.0 / (10000.0 ** (np.arange(0, 64, 2, dtype=np.float64) / 64))
    t = np.arange(S, dtype=np.float64)
    emb = np.concatenate([np.outer(t, inv)] * 2, -1)          # [S,64]
    cos_t = np.cos(emb).T.astype(np.float32)                  # [64,S]
    sin_t = np.sin(emb).T.astype(np.float32)
    ssin = np.concatenate([-sin_t[:32], sin_t[32:]], 0)
    cos_tile = np.ascontiguousarray(np.vstack([cos_t, cos_t])).astype(BF16)
    nsin_tile = np.ascontiguousarray(np.vstack([ssin, ssin])).astype(BF16)
    return cos_tile, nsin_tile


def kernel(x, Wq, Wk, Wv, Wo):
    x = np.asarray(x, np.float32)
    Wq, Wk, Wv, Wo = (np.asarray(w, np.float32) for w in (Wq, Wk, Wv, Wo))
    if "nc" not in _CACHE:
        _CACHE["nc"] = _build()
    nc = _CACHE["nc"]
    cos_tile, nsin_tile = _tables()
    ident = np.eye(128, dtype=BF16)
    xts = [np.ascontiguousarray(x[b].T).astype(BF16) for b in range(B)]
    in_maps = []
    for core in range(N_CORES):
        b, g = divmod(core, 4)
        fsl = slice(FH * g, FH * (g + 1))
        dsl = slice(128 * g, 128 * (g + 1))
        in_maps.append({
            "xt": xts[b],
            "wqt": np.ascontiguousarray(Wq[fsl].T).astype(BF16),
            "wkt": np.ascontiguousarray(Wk[dsl].T).astype(BF16),
            "wvt": np.ascontiguousarray(Wv[dsl].T).astype(BF16),
            "wot": np.ascontiguousarray(Wo[:, fsl].T).astype(BF16),
            "cost": cos_tile,
            "nsint": nsin_tile,
            "ident": ident,
        })
    res = run_bass_kernel_spmd(nc, in_maps, core_ids=list(range(N_CORES)),
                               **_CACHE.get("run_kwargs", {}))
    _CACHE["last_result"] = res
    out = np.empty((B, S, E), np.float32)
    for b in range(B):
        out[b] = sum(res.results[4 * b + g]["out"] for g in range(4))
    return out
